# revision 1
# baseline (speedup 1.0000x reference)
"""GQA sliding-window attention (training path, no causal mask, no 1/sqrt(d)
scaling) on 8 Trainium2 NeuronCores.

Reference semantics (see original nn.Module):
  q = x@Wq+bq [b,s,16,64]; k,v = x@Wk+bk / x@Wv+bv [b,s,2,64]
  k,v zero-padded by 128 on both sides of s; query i attends padded
  positions [i, i+256) (i.e. global [i-128, i+128)); padded positions
  contribute score 0 (exp->1) and value 0. out = attn @ Wo + bo.

Sharding: batch x sequence. 8 shards = 2 batches x 4 chunks of 512 query
rows. Each core receives x^T for its 512 rows plus a 128-row halo on each
side (zero rows outside [0, 2048)), with an appended 0/1 validity row so
that K/V bias is only added at in-range positions (k = x@Wk + valid*bk).
Host gathers/concatenates per-core outputs; no collectives.

Per-core dataflow (all fp32; matmuls issued as float32r):
  xT -> qT [dk, s] / kT / vT projections (PE, contraction over 1024)
  vT transposed back to V [w, dk] via PE transpose; a ones-column is
  appended to V so each PV matmul also produces the softmax denominator.
  Scores computed transposed, S^T[w, q] = kT^T qT, per 128-wide kv chunk
  (6 chunks cover the 768 halo), q-window 384 per chunk.
  exp on ScalarE (one batched op per 3 chunks), band masking via
  GPSIMD affine_select triangles, PV accumulated over chunks into a
  [65, 512] PSUM tile (row 64 = denominator). Normalization multiplies
  by an outer-product broadcast of 1/denom, then the Wo projection.
"""

import numpy as np

DIM = 1024
NH = 16  # query heads
G = 2  # kv heads
HD = 64  # head dim
W = 256  # window
HALF = 128
BATCH, SEQ = 2, 2048
NCORES = 8
SQ = 512  # query rows per core
SK = SQ + 2 * HALF  # 768 kv halo rows per core
KC = DIM // 128  # 8 contraction chunks
NJ = SK // 128  # 6 kv chunks
SP = 384  # score q-window width per kv chunk

# per-chunk PV accumulation window [lo, hi) in local q coords, and the
# q-offset ws of the chunk's 384-wide score window
# chunk j's PV accumulation window [lo, hi) in local q coords. Edge chunks
# 0/5 are widened to 256 so every PV matmul has N>=256 (1 cycle/row f32r);
# the widened region is zeroed by the same affine_select that cuts the
# triangle, so the extra columns contribute nothing.
PV_WIN = {0: (0, 256), 1: (0, 256), 2: (0, 384), 3: (128, 512), 4: (256, 512), 5: (256, 512)}
WS = {j: (0 if j < 3 else 128) for j in range(NJ)}
# PV issue order: j1 [0,256) and j4 [256,512) partition the PSUM zero
# region exactly, so every byte is written once before any accumulation
# (has_written zero-region semantics); stop on the last.
PV_ORDER = [1, 4, 0, 2, 3, 5]

_CACHE = {}


def _build_program(dbg=False):
    import concourse.bass as bass
    import concourse.mybir as mybir
    import concourse.tile as tile
    from concourse import bacc

    f32 = mybir.dt.float32
    f32r = mybir.dt.float32r

    nc = bacc.Bacc("TRN2", target_bir_lowering=False, debug=False, num_devices=NCORES)
    dbg_t = {}
    if dbg:
        for name, shape in [
            ("dbg_qT", [128, KC, SQ]), ("dbg_kT", [128, SK]), ("dbg_vT", [128, SK]),
            ("dbg_vt", [128, NJ, G, HD + 1]), ("dbg_pt0", [128, NJ, SP]),
            ("dbg_pt8", [128, NJ, SP]),
            ("dbg_attnT", [128, KC, SQ]),
            ("dbg_attnN", [128, KC, SQ]),
        ]:
            dbg_t[name] = nc.declare_dram_parameter(name, shape, f32, isOutput=True)

    xaT = nc.declare_dram_parameter("xaT", [DIM + 1, SK], f32r, isOutput=False)
    wq = nc.declare_dram_parameter("wq", [DIM, DIM], f32r, isOutput=False)
    wk = nc.declare_dram_parameter("wk", [DIM + 1, G * HD], f32r, isOutput=False)
    wv = nc.declare_dram_parameter("wv", [DIM + 1, G * HD], f32r, isOutput=False)
    wo = nc.declare_dram_parameter("wo", [DIM, DIM], f32r, isOutput=False)
    bq = nc.declare_dram_parameter("bq", [DIM, 1], f32, isOutput=False)
    bo = nc.declare_dram_parameter("bo", [DIM, 1], f32, isOutput=False)
    sel2 = nc.declare_dram_parameter("sel2", [128, 128], f32r, isOutput=False)
    identD = nc.declare_dram_parameter("ident", [128, 128], f32r, isOutput=False)
    ones2 = nc.declare_dram_parameter("ones2", [128, G], f32r, isOutput=False)
    yT = nc.declare_dram_parameter("yT", [DIM, SQ], f32, isOutput=True)

    def r(ap):
        return ap

    with tile.TileContext(nc) as tc:
        with (
            nc.allow_low_precision("fp32r (tf32) matmul inputs; accumulation stays fp32"),
            tc.tile_pool(name="wts", bufs=1) as wts,
            tc.tile_pool(name="sb", bufs=1) as sb,
            tc.tile_pool(name="pt", bufs=3) as ptp,
            tc.tile_pool(name="yst", bufs=2) as yst,
            tc.tile_pool(name="psA", bufs=3, space="PSUM") as psA,
            tc.tile_pool(name="pvP", bufs=2, space="PSUM") as pvP,
        ):
            # ---- constant loads ----
            # Big streams split across the sync and scalar HWDGE queues
            # (even/odd chunks) in compute order: xT -> wq -> wo. All small
            # loads ride the GPSIMD SWDGE queue, consolidated into single
            # strided DMAs, so neither big queue nor the ScalarE sequencer
            # is clogged at attention start.
            xT_sb = wts.tile([128, KC, SK], f32r, tag="xT")
            wq_sb = wts.tile([128, KC, DIM], f32r, tag="wq")
            wo_sb = wts.tile([128, KC, DIM], f32r, tag="wo")
            for kc in range(KC):
                eng = nc.sync if kc % 2 == 0 else nc.scalar
                eng.dma_start(out=xT_sb[:, kc, :], in_=xaT[kc * 128:(kc + 1) * 128, :])
            xaug = wts.tile([1, SK], f32r, tag="xaug")
            nc.gpsimd.dma_start(out=xaug[:, :], in_=xaT[DIM:DIM + 1, :])
            wk_sb = wts.tile([128, KC, G * HD], f32r, tag="wk")
            wv_sb = wts.tile([128, KC, G * HD], f32r, tag="wv")
            for kc in range(KC):
                nc.sync.dma_start(out=wk_sb[:, kc, :], in_=wk[kc * 128:(kc + 1) * 128, :])
                nc.scalar.dma_start(out=wv_sb[:, kc, :], in_=wv[kc * 128:(kc + 1) * 128, :])
            for kc in range(KC):
                eng = nc.sync if kc % 2 == 0 else nc.scalar
                eng.dma_start(out=wq_sb[:, kc, :], in_=wq[kc * 128:(kc + 1) * 128, :])
            for kc in range(KC):
                eng = nc.sync if kc % 2 == 0 else nc.scalar
                eng.dma_start(out=wo_sb[:, kc, :], in_=wo[kc * 128:(kc + 1) * 128, :])
            wk_aug = wts.tile([1, G * HD], f32r, tag="wkaug")
            wv_aug = wts.tile([1, G * HD], f32r, tag="wvaug")
            nc.gpsimd.dma_start(out=wk_aug[:, :], in_=wk[DIM:DIM + 1, :])
            nc.gpsimd.dma_start(out=wv_aug[:, :], in_=wv[DIM:DIM + 1, :])

            bq_sb = wts.tile([128, KC], f32, tag="bq")
            bo_sb = wts.tile([128, KC], f32, tag="bo")
            nc.gpsimd.dma_start(
                out=bq_sb[:, :], in_=bq.rearrange("(a p) c -> p (a c)", p=128))
            nc.gpsimd.dma_start(
                out=bo_sb[:, :], in_=bo.rearrange("(a p) c -> p (a c)", p=128))
            sel2_sb = wts.tile([128, 128], f32r, tag="sel2")
            nc.gpsimd.dma_start(out=sel2_sb[:, :], in_=sel2[:, :])

            ident = wts.tile([128, 128], f32r, tag="ident")
            nc.gpsimd.dma_start(out=ident[:, :], in_=identD[:, :])
            ones_sb = wts.tile([128, G], f32r, tag="ones")
            nc.gpsimd.dma_start(out=ones_sb[:, :], in_=ones2[:, :])

            # ---- persistent intermediates ----
            qT_sb = sb.tile([128, KC, SQ], f32r, tag="qT")     # [dk(2 heads), dd, q]
            kT_sb = sb.tile([128, SK], f32r, tag="kT")         # [dk(2 groups), w]
            vT_sb = sb.tile([128, SK], f32r, tag="vT")
            vt_t = [
                sb.tile([128, G, HD + 1], f32r, tag=f"vt{j}", name=f"vt{j}")
                for j in range(NJ)
            ]
            attnT = sb.tile([128, KC, SQ], f32r, tag="attnT")  # [dk(2 heads), pair, q]
            # denominators at 32-partition strides: pair slot k=p%4 puts
            # head (p,g=0) on partition 32k (legal DVE write base) and
            # head (p,g=1) on 32k+1 (written via DMA bounce); reciprocal
            # then engages 128 lanes at free-size 512.
            den = sb.tile([128, 3, SQ], f32, tag="den")
            den_r = sb.tile([128, 3, SQ], f32r, tag="denr")
            nc.vector.memset(den[:, :, :], 1.0)

            # ---- K/V projections over the full 768 halo (+ aug bias row) ----
            for (wmat, waug, dst) in ((wk_sb, wk_aug, kT_sb), (wv_sb, wv_aug, vT_sb)):
                for h2 in range(2):
                    ps = psA.tile([128, 2, 512], f32, tag="ps")
                    out = ps[:, 0, 0:SP]
                    sl = slice(h2 * SP, (h2 + 1) * SP)
                    for kc in range(KC):
                        nc.tensor.matmul(
                            out, r(wmat[:, kc, :]), r(xT_sb[:, kc, sl]),
                            start=(kc == 0), stop=False,
                        )
                    nc.tensor.matmul(out, r(waug[:, :]), r(xaug[:, sl]),
                                     start=False, stop=True)
                    nc.vector.tensor_copy(dst[:, sl], out)

            # ---- V back to natural layout [w, dk], ones column appended ----
            for j in range(NJ):
                ps = psA.tile([128, 2, 512], f32r, tag="ps", name=f"pstr{j}")
                out = ps[:, 0, 0:128]
                nc.tensor.transpose(out, vT_sb[:, j * 128:(j + 1) * 128], ident)
                nc.vector.tensor_copy(
                    vt_t[j][:, :, 0:HD],
                    out.rearrange("p (g d) -> p g d", g=G),
                )
                nc.vector.tensor_copy(vt_t[j][:, :, HD:HD + 1], ones_sb[:, :])

            if dbg:
                nc.sync.dma_start(out=dbg_t["dbg_qT"][:, :, :], in_=qT_sb[:, :, :])
                nc.sync.dma_start(out=dbg_t["dbg_kT"][:, :], in_=kT_sb[:, :])
                nc.sync.dma_start(out=dbg_t["dbg_vT"][:, :], in_=vT_sb[:, :])
                for j in range(NJ):
                    nc.sync.dma_start(out=dbg_t["dbg_vt"][:, j, :, :], in_=vt_t[j][:, :, :])

            # ---- attention per head, normalization interleaved per wave ----
            # Host permutes Wq columns so q dd-block p holds head p (group 0)
            # in rows 0:64 and head p+8 (group 1) in rows 64:128 — score
            # matmul operands then share a base partition with kT's groups.
            def norm_recip(c3):
                # reciprocal for wave c3 (pairs 3*c3 .. 3*c3+2); emitted as
                # soon as the wave's denominators are complete so the
                # PE-side normalization never waits on the DVE
                nc.vector.reciprocal(den_r[:, c3, :], den[:, c3, :])

            def norm_apply(plo, phi):
                for p in range(plo, phi):
                    ps = psA.tile([128, 2, 512], f32, tag="ps", name=f"psn{p}")
                    rb = ps[:, 0, :]
                    k4 = 32 * (p % 3)
                    nc.tensor.matmul(rb, r(sel2_sb[k4:k4 + 2, :]),
                                     r(den_r[k4:k4 + 2, p // 3, :]),
                                     start=True, stop=True)
                    nc.vector.tensor_mul(attnT[:, p, :], attnT[:, p, :], rb)

            def q_proj(dd):
                # Q projection block dd: qT[dd] = (Wq^T x^T)[dd] + bq.
                # Emitted just-in-time inside the attention loop so the PE
                # stream stays dense (512-wide projection matmuls fill the
                # gaps between attention groups and keep the clock un-gated).
                ps = psA.tile([128, 2, 512], f32, tag="ps", name=f"psq{dd}")
                out = ps[:, 0, :]
                for kc in range(KC):
                    nc.tensor.matmul(
                        out, r(wq_sb[:, kc, dd * 128:(dd + 1) * 128]),
                        r(xT_sb[:, kc, HALF:HALF + SQ]),
                        start=(kc == 0), stop=(kc == KC - 1),
                    )
                nc.scalar.activation(
                    qT_sb[:, dd, :], out, mybir.ActivationFunctionType.Identity,
                    bias=bq_sb[:, dd:dd + 1],
                )

            q_proj(0)
            for p_g in [(p, gg) for p in range(KC) for gg in range(G)]:
                p, gg = p_g
                if gg == 0 and p < KC - 1:
                    q_proj(p + 1)
                if (p, gg) == (3, 0):
                    norm_recip(0)
                if (p, gg) == (5, 0):
                    norm_apply(0, 3)
                if (p, gg) == (6, 0):
                    norm_recip(1)
                if (p, gg) == (7, 0):
                    # pair 6's reciprocal (wave-2 rows 0:2) overlaps pair 7
                    nc.vector.reciprocal(den_r[0:2, 2, :], den[0:2, 2, :])
                if (p, gg) == (7, 1):
                    norm_apply(3, 7)
                h = p + 8 * gg
                g = gg
                qrow = 64 * gg
                qT_h = qT_sb[qrow:qrow + 64, p, :]
                psc = [
                    psA.tile([128, 2, 512], f32, tag="ps", name=f"psc{h}_{i}")
                    for i in range(3)
                ]
                for j in range(NJ):
                    ws = WS[j]
                    nc.tensor.matmul(
                        psc[j // 2][:, j % 2, 0:SP],
                        r(kT_sb[64 * g:64 * g + 64, j * 128:(j + 1) * 128]),
                        r(qT_h[:, ws:ws + SP]),
                        start=True, stop=True,
                    )
                pt = ptp.tile([128, NJ, SP], f32r, tag="pt")
                # exp only the columns the PV windows read: thirds cover
                # chunk pairs (0,1): cols [0,256), (2,3): [0,384),
                # (4,5): [128,384)
                for c3, (elo, ehi) in enumerate(((0, 256), (0, SP), (128, SP))):
                    nc.scalar.activation(pt[:, 2 * c3:2 * c3 + 2, elo:ehi],
                                         psc[c3][:, :, elo:ehi],
                                         mybir.ActivationFunctionType.Exp)

                # band masking: keep iff 0 <= (128j + ww) - q < 256.
                # The upper-bound select on chunk 0 spans cols [0,256) (its
                # widened PV window): cols >= 128 fail the condition for
                # every partition and are filled with 0. Likewise the
                # lower-bound select on chunk 5.
                for j in range(NJ):
                    lo, hi = PV_WIN[j]
                    ws = WS[j]
                    cb = lo
                    while cb < hi:
                        wdt = 256 if (j in (0, 5) and cb == (0 if j == 0 else 256)) else 128
                        c0 = cb - ws
                        region = pt[:, j, c0:c0 + wdt]
                        if cb > 128 * (j - 1):  # upper bound: q <= 128j + ww
                            nc.gpsimd.affine_select(
                                out=region, in_=region,
                                compare_op=mybir.AluOpType.is_ge, fill=0.0,
                                base=128 * j - cb, channel_multiplier=1,
                                pattern=[[-1, wdt]],
                            )
                        elif cb < 128 * (j - 1):  # lower: q > 128j + ww - 256
                            nc.gpsimd.affine_select(
                                out=region, in_=region,
                                compare_op=mybir.AluOpType.is_ge, fill=0.0,
                                base=cb - 128 * j + 255, channel_multiplier=-1,
                                pattern=[[1, wdt]],
                            )
                        cb += wdt

                if dbg and h in (0, 8):
                    nc.sync.dma_start(out=dbg_t[f"dbg_pt{h}"][:, :, :], in_=pt[:, :, :])
                pv = pvP.tile([128, 512], f32, tag="pv")
                for j in PV_ORDER:
                    lo, hi = PV_WIN[j]
                    ws = WS[j]
                    nc.tensor.matmul(
                        pv[0:HD + 1, lo:hi],
                        r(vt_t[j][:, g, :]),
                        r(pt[:, j, lo - ws:hi - ws]),
                        start=(j == PV_ORDER[0]), stop=(j == PV_ORDER[-1]),
                    )
                nc.vector.tensor_copy(attnT[qrow:qrow + 64, p, :], pv[0:HD, :])
                if gg == 0:
                    nc.vector.tensor_copy(den[32 * (p % 3):32 * (p % 3) + 1, p // 3, :], pv[HD:HD + 1, :])
                else:
                    # engine writes must start at partition 0/32/64/96; bounce
                    # through partition 0 and DMA into den partition 1
                    dtmp = yst.tile([1, SQ], f32, tag="dtmp", name=f"dtmp{h}")
                    nc.vector.tensor_copy(dtmp[:, :], pv[HD:HD + 1, :])
                    nc.sync.dma_start(out=den[32 * (p % 3) + 1:32 * (p % 3) + 2, p // 3, :], in_=dtmp[:, :])

            if dbg:
                nc.sync.dma_start(out=dbg_t["dbg_attnT"][:, :, :], in_=attnT[:, :, :])

            nc.vector.reciprocal(den_r[32:34, 2, :], den[32:34, 2, :])
            norm_apply(7, 8)

            if dbg:
                nc.sync.dma_start(out=dbg_t["dbg_attnN"][:, :, :], in_=attnT[:, :, :])

            # ---- output projection ----
            for do in range(KC):
                ps = psA.tile([128, 2, 512], f32, tag="ps")
                out = ps[:, 0, :]
                for p in range(KC):
                    nc.tensor.matmul(
                        out, r(wo_sb[:, p, do * 128:(do + 1) * 128]),
                        r(attnT[:, p, :]),
                        start=(p == 0), stop=(p == KC - 1),
                    )
                yt = yst.tile([128, SQ], f32, tag="yt")
                nc.scalar.activation(yt, out, mybir.ActivationFunctionType.Identity,
                                     bias=bo_sb[:, do:do + 1])
                eng = nc.sync if do % 2 == 0 else nc.scalar
                eng.dma_start(out=yT[do * 128:(do + 1) * 128, :], in_=yt[:, :])

    nc.finalize()
    return nc


def get_program():
    if "nc" not in _CACHE:
        _CACHE["nc"] = _build_program()
    return _CACHE["nc"]


def make_in_maps(x, Wq, bq, Wk, bk, Wv, bv, Wo, bo):
    """Host-side sharding: per-core input dicts."""
    x = np.ascontiguousarray(np.asarray(x, np.float32))
    wkb = np.concatenate([np.asarray(Wk, np.float32), np.asarray(bk, np.float32)[None]], 0)
    wvb = np.concatenate([np.asarray(Wv, np.float32), np.asarray(bv, np.float32)[None]], 0)
    sel2 = np.zeros((128, 128), np.float32)
    sel2[0::32, :64] = 1.0
    sel2[1::32, 64:] = 1.0
    # head permutation: device column-block p holds [head p | head p+8]
    # (so each q dd-block pairs a group-0 head with a group-1 head at
    # matching base partitions). perm maps device attn-dim -> original dim.
    perm = np.empty(DIM, np.int64)
    for p in range(8):
        perm[128 * p:128 * p + 64] = np.arange(64 * p, 64 * p + 64)
        perm[128 * p + 64:128 * p + 128] = np.arange(64 * (p + 8), 64 * (p + 8) + 64)
    common = {
        "wq": np.ascontiguousarray(np.asarray(Wq, np.float32)[:, perm]),
        "wk": np.ascontiguousarray(wkb),
        "wv": np.ascontiguousarray(wvb),
        "wo": np.ascontiguousarray(np.asarray(Wo, np.float32)[perm, :]),
        "bq": np.ascontiguousarray(np.asarray(bq, np.float32)[perm].reshape(DIM, 1)),
        "bo": np.ascontiguousarray(np.asarray(bo, np.float32).reshape(DIM, 1)),
        "sel2": sel2,
        "ident": np.eye(128, dtype=np.float32),
        "ones2": np.ones((128, G), np.float32),
    }
    in_maps = []
    for c in range(NCORES):
        b, t = divmod(c, NCORES // BATCH)
        s0 = SQ * t
        xa = np.zeros((SK, DIM + 1), np.float32)
        lo, hi = max(0, s0 - HALF), min(SEQ, s0 + SQ + HALF)
        xa[lo - (s0 - HALF):hi - (s0 - HALF), :DIM] = x[b, lo:hi]
        xa[lo - (s0 - HALF):hi - (s0 - HALF), DIM] = 1.0
        in_maps.append({"xaT": np.ascontiguousarray(xa.T), **common})
    return in_maps


def assemble_output(results):
    y = np.empty((BATCH, SEQ, DIM), np.float32)
    for c in range(NCORES):
        b, t = divmod(c, NCORES // BATCH)
        y[b, SQ * t:SQ * (t + 1), :] = results[c]["yT"].T
    return y


def kernel(**inputs):
    from concourse.bass_utils import run_bass_kernel_spmd

    nc = get_program()
    in_maps = make_in_maps(**inputs)
    last_err = None
    for _ in range(3):  # retry: transient NRT device wedges recover on rerun
        try:
            res = run_bass_kernel_spmd(nc, in_maps, list(range(NCORES)))
            return assemble_output(res.results)
        except Exception as e:  # noqa: BLE001
            last_err = e
    raise last_err



# revision 10
# speedup vs baseline: 1.1504x; 1.1504x over previous
"""GQA sliding-window attention (training path, no causal mask, no 1/sqrt(d)
scaling) on 8 Trainium2 NeuronCores.

Reference semantics (see original nn.Module):
  q = x@Wq+bq [b,s,16,64]; k,v = x@Wk+bk / x@Wv+bv [b,s,2,64]
  k,v zero-padded by 128 on both sides of s; query i attends padded
  positions [i, i+256) (i.e. global [i-128, i+128)); padded positions
  contribute score 0 (exp->1) and value 0. out = attn @ Wo + bo.

Sharding: batch x sequence. 8 shards = 2 batches x 4 chunks of 512 query
rows. Each core receives x^T for its 512 rows plus a 128-row halo on each
side (zero rows outside [0, 2048)), with an appended 0/1 validity row so
that K/V bias is only added at in-range positions. Host gathers per-core
outputs; no collectives.

Per-core dataflow (bf16 matmul inputs, fp32 PSUM accumulation):
  xT -> kT/vT projections (bias via augmented weight row), V transposed
  back to natural layout with a ones column appended (so PV matmuls also
  produce the softmax denominator). qT computed per 128-dim block with
  ScalarE bias-add; host permutes Wq columns so block p holds head p
  (group 0) in rows 0:64 and head p+8 (group 1) in rows 64:128.
  Attention runs in 4-head-packed tiles: per (group, head-half, qblock),
  scores S^T[w 128, 4 heads, 128 q] are one N=512 matmul per window
  chunk (3 chunks cover each qblock's 384-position window), exp on
  ScalarE, band-mask triangles via 2 GPSIMD affine_selects (multi-dim
  pattern broadcasts over the 4 heads), PV accumulates [65, 4, 128]
  (row 64 = denominator). Denominator reciprocal via DVE
  reciprocal_approx_fast directly from PSUM; normalization multiplies
  attnT by the reciprocal broadcast across partitions (stride-0 AP, or
  a ones-outer-product PSUM matmul fallback). Output projection with
  ScalarE bias-add, streamed to DRAM.
"""

import numpy as np

DIM = 1024
NH = 16  # query heads
G = 2  # kv heads
HD = 64  # head dim
W = 256  # window
HALF = 128
BATCH, SEQ = 2, 2048
NCORES = 8
SQ = 512  # query rows per core
SK = SQ + 2 * HALF  # 768 kv halo rows per core
KC = DIM // 128  # 8 contraction chunks
NJ = SK // 128  # 6 kv chunks

USE_PBCAST = False  # stride-0 partition APs are rejected by the DVE lowering

_CACHE = {}


def _build_program(dbg=False):
    import concourse.bass as bass
    import concourse.mybir as mybir
    import concourse.tile as tile
    from concourse import bacc

    f32 = mybir.dt.float32
    f32r = mybir.dt.float32r
    bf16 = mybir.dt.bfloat16

    nc = bacc.Bacc("TRN2", target_bir_lowering=False, debug=False, num_devices=NCORES)
    dbg_t = {}
    if dbg:
        for name, shape, dt in [
            ("dbg_qT", [128, KC, SQ], bf16), ("dbg_kT", [128, SK], bf16),
            ("dbg_vT", [128, SK], f32r),
            ("dbg_pt0", [128, 3, 4, 128], bf16),
            ("dbg_attnT", [128, KC, SQ], bf16),
            ("dbg_den", [1, 4, 128], f32r),
            ("dbg_pvden", [1, 4, 128], f32),
            ("dbg_recs", [1, 4, 128], f32),
        ]:
            dbg_t[name] = nc.declare_dram_parameter(name, shape, dt, isOutput=True)

    xaT = nc.declare_dram_parameter("xaT", [DIM + 1, SK], bf16, isOutput=False)
    wq = nc.declare_dram_parameter("wq", [KC, 128, KC, 128], bf16, isOutput=False)
    wk = nc.declare_dram_parameter("wk", [DIM + 1, G * HD], bf16, isOutput=False)
    wv = nc.declare_dram_parameter("wv", [DIM + 1, G * HD], bf16, isOutput=False)
    wo = nc.declare_dram_parameter("wo", [KC, 128, KC, 128], bf16, isOutput=False)
    bq = nc.declare_dram_parameter("bq", [DIM, 1], f32, isOutput=False)
    bo = nc.declare_dram_parameter("bo", [DIM, 1], f32, isOutput=False)
    identD = nc.declare_dram_parameter("ident", [128, 128], f32r, isOutput=False)
    ones2 = nc.declare_dram_parameter("ones2", [128, G], bf16, isOutput=False)
    ones64 = nc.declare_dram_parameter("ones64", [1, 64], f32r, isOutput=False)
    yT = nc.declare_dram_parameter("yT", [DIM, SQ], f32, isOutput=True)

    with tile.TileContext(nc) as tc:
        with (
            nc.allow_low_precision("bf16 matmul inputs; accumulation stays fp32"),
            tc.tile_pool(name="wts", bufs=1) as wts,
            tc.tile_pool(name="sb", bufs=1) as sb,
            tc.tile_pool(name="pt", bufs=2) as ptp,
            tc.tile_pool(name="dr", bufs=3) as drp,
            tc.tile_pool(name="yst", bufs=2) as yst,
            tc.tile_pool(name="psA", bufs=3, space="PSUM") as psA,
            tc.tile_pool(name="psS", bufs=2, space="PSUM") as psS,
            tc.tile_pool(name="pvP", bufs=3, space="PSUM") as pvP,
        ):
            # ---- constant loads (critical-path order: wk/wv, xT, wq, wo) ----
            wk_sb = wts.tile([128, KC, G * HD], bf16, tag="wk")
            wv_sb = wts.tile([128, KC, G * HD], bf16, tag="wv")
            nc.sync.dma_start(
                out=wk_sb[:, :, :],
                in_=wk[0:DIM, :].rearrange("(kc p) m -> p kc m", p=128))
            nc.scalar.dma_start(
                out=wv_sb[:, :, :],
                in_=wv[0:DIM, :].rearrange("(kc p) m -> p kc m", p=128))
            xT_sb = wts.tile([128, KC, SK], bf16, tag="xT")
            for kc in range(KC):
                eng = nc.sync if kc % 2 == 0 else nc.scalar
                eng.dma_start(out=xT_sb[:, kc, :], in_=xaT[kc * 128:(kc + 1) * 128, :])
            wq_sb = wts.tile([128, KC, KC, 128], bf16, tag="wq")
            for dd in range(KC):
                eng = nc.sync if dd % 2 == 0 else nc.scalar
                eng.dma_start(out=wq_sb[:, dd, :, :], in_=wq[dd, :, :, :])
            wo_sb = wts.tile([128, KC, KC, 128], bf16, tag="wo")
            for do in range(KC):
                eng = nc.sync if do % 2 == 0 else nc.scalar
                eng.dma_start(out=wo_sb[:, do, :, :], in_=wo[do, :, :, :])

            xaug = wts.tile([1, SK], bf16, tag="xaug")
            nc.gpsimd.dma_start(out=xaug[:, :], in_=xaT[DIM:DIM + 1, :])
            wk_aug = wts.tile([1, G * HD], bf16, tag="wkaug")
            wv_aug = wts.tile([1, G * HD], bf16, tag="wvaug")
            nc.gpsimd.dma_start(out=wk_aug[:, :], in_=wk[DIM:DIM + 1, :])
            nc.gpsimd.dma_start(out=wv_aug[:, :], in_=wv[DIM:DIM + 1, :])
            bq_sb = wts.tile([128, KC], f32, tag="bq")
            bo_sb = wts.tile([128, KC], f32, tag="bo")
            nc.gpsimd.dma_start(
                out=bq_sb[:, :], in_=bq.rearrange("(a p) c -> p (a c)", p=128))
            nc.gpsimd.dma_start(
                out=bo_sb[:, :], in_=bo.rearrange("(a p) c -> p (a c)", p=128))
            ident = wts.tile([128, 128], f32r, tag="ident")
            nc.gpsimd.dma_start(out=ident[:, :], in_=identD[:, :])
            ones_sb = wts.tile([128, G], bf16, tag="ones")
            nc.gpsimd.dma_start(out=ones_sb[:, :], in_=ones2[:, :])
            ones64_sb = wts.tile([1, 64], f32r, tag="ones64")
            nc.gpsimd.dma_start(out=ones64_sb[:, :], in_=ones64[:, :])

            # ---- persistent intermediates ----
            qT_sb = sb.tile([128, KC, SQ], bf16, tag="qT")  # [dk(2 heads), dd, q]
            kT_sb = sb.tile([128, SK], bf16, tag="kT")      # [dk(2 groups), w]
            vT_sb = sb.tile([128, SK], f32r, tag="vT")
            vt_t = [
                sb.tile([128, G, HD + 1], bf16, tag=f"vt{j}", name=f"vt{j}")
                for j in range(NJ)
            ]
            attnT = sb.tile([128, KC, SQ], bf16, tag="attnT")  # [dk(2 heads), pair, q]

            # ---- K/V projections over the full 768 halo (+ aug bias row) ----
            for (wmat, waug, dst) in ((wk_sb, wk_aug, kT_sb), (wv_sb, wv_aug, vT_sb)):
                for h2 in range(2):
                    ps = psA.tile([128, 512], f32, tag="ps")
                    out = ps[:, 0:384]
                    sl = slice(h2 * 384, (h2 + 1) * 384)
                    for kc in range(KC):
                        nc.tensor.matmul(
                            out, wmat[:, kc, :], xT_sb[:, kc, sl],
                            start=(kc == 0), stop=False,
                        )
                    nc.tensor.matmul(out, waug[:, :], xaug[:, sl],
                                     start=False, stop=True)
                    nc.vector.tensor_copy(dst[:, sl], out)

            # ---- V back to natural layout [w, dk], ones column appended ----
            for j in range(NJ):
                ps = psA.tile([128, 512], f32r, tag="ps", name=f"pstr{j}")
                out = ps[:, 0:128]
                nc.tensor.transpose(out, vT_sb[:, j * 128:(j + 1) * 128], ident)
                nc.vector.tensor_copy(
                    vt_t[j][:, :, 0:HD],
                    out.rearrange("p (g d) -> p g d", g=G),
                )
                nc.vector.tensor_copy(vt_t[j][:, :, HD:HD + 1], ones_sb[:, :])

            def q_proj(dd):
                ps = psA.tile([128, 512], f32, tag="ps", name=f"psq{dd}")
                out = ps[:, :]
                for kc in range(KC):
                    nc.tensor.matmul(
                        out, wq_sb[:, dd, kc, :], xT_sb[:, kc, HALF:HALF + SQ],
                        start=(kc == 0), stop=(kc == KC - 1),
                    )
                nc.scalar.activation(
                    qT_sb[:, dd, :], out, mybir.ActivationFunctionType.Identity,
                    bias=bq_sb[:, dd:dd + 1],
                )

            if dbg:
                nc.sync.dma_start(out=dbg_t["dbg_kT"][:, :], in_=kT_sb[:, :])
                nc.sync.dma_start(out=dbg_t["dbg_vT"][:, :], in_=vT_sb[:, :])

            # ---- attention in 4-head-packed tiles ----
            # Tile (g, hh, qb): heads 4hh..4hh+3 of group g, local q block
            # [128qb, 128qb+128). Window chunks qb+c (c=0..2); band masks:
            # c=0 keeps ww-qq>=0, c=2 keeps qq-ww-1>=0, c=1 is full.
            for dd in range(4):
                q_proj(dd)

            tiles = [(hh, g, qb) for hh in range(2) for g in range(G)
                     for qb in range(4)]
            pending = None

            def emit_norm(p):
                g, hh, qb, dr = p
                region = attnT[64 * g:64 * g + 64, 4 * hh:4 * hh + 4,
                               qb * 128:(qb + 1) * 128]
                if USE_PBCAST:
                    rb = dr[0:1, :, :].partition_broadcast(64)
                    if rb.ndim == 4:
                        rb = rb.squeeze(1)
                    nc.vector.tensor_mul(region, region, rb)
                else:
                    rbp = psA.tile([128, 512], f32, tag="ps",
                                   name=f"rb{g}_{hh}_{qb}")
                    nc.tensor.matmul(rbp[0:64, :], ones64_sb[:, :],
                                     dr[:, :, :],
                                     start=True, stop=True)
                    nc.vector.tensor_mul(
                        region, region,
                        rbp[0:64, :].rearrange("p (a b) -> p a b", a=4))

            for ti, (hh, g, qb) in enumerate(tiles):
                pt = ptp.tile([128, 3, 4, 128], bf16, tag="pt")
                for c in range(3):
                    ps = psS.tile([128, 4, 128], f32, tag="sc")
                    nc.tensor.matmul(
                        ps[:, :, :],
                        kT_sb[64 * g:64 * g + 64, (qb + c) * 128:(qb + c + 1) * 128],
                        qT_sb[64 * g:64 * g + 64, 4 * hh:4 * hh + 4,
                              qb * 128:(qb + 1) * 128],
                        start=True, stop=True,
                    )
                    nc.scalar.activation(pt[:, c, :, :], ps[:, :, :],
                                         mybir.ActivationFunctionType.Exp)
                    if c == 0:
                        nc.gpsimd.affine_select(
                            out=pt[:, 0, :, :], in_=pt[:, 0, :, :],
                            compare_op=mybir.AluOpType.is_ge, fill=0.0,
                            base=0, channel_multiplier=1,
                            pattern=[[0, 4], [-1, 128]],
                        )
                    elif c == 2:
                        nc.gpsimd.affine_select(
                            out=pt[:, 2, :, :], in_=pt[:, 2, :, :],
                            compare_op=mybir.AluOpType.is_ge, fill=0.0,
                            base=-1, channel_multiplier=-1,
                            pattern=[[0, 4], [1, 128]],
                        )
                # JIT q-projection for the second head-half while the first
                # half's tiles run, keeping the PE stream dense.
                if ti < 4:
                    q_proj(4 + ti)
                if dbg and (hh, g, qb) == (0, 0, 0):
                    nc.sync.dma_start(out=dbg_t["dbg_pt0"][:, :, :, :],
                                      in_=pt[:, :, :, :])
                pv = pvP.tile([128, 4, 128], f32, tag="pv")
                for c in range(3):
                    nc.tensor.matmul(
                        pv[0:HD + 1, :, :],
                        vt_t[qb + c][:, g, :],
                        pt[:, c, :, :],
                        start=(c == 0), stop=(c == 2),
                    )
                if pending is not None:
                    emit_norm(pending)
                nc.vector.tensor_copy(
                    attnT[64 * g:64 * g + 64, 4 * hh:4 * hh + 4,
                          qb * 128:(qb + 1) * 128],
                    pv[0:HD, :, :])
                # reciprocal_approx_fast cannot read PSUM: ScalarE moves the
                # denominator row to SBUF, DVE approximates 1/x there, and a
                # second ScalarE Identity provides the f32r rounding the rb
                # matmul input requires.
                dn = drp.tile([1, 4, 128], f32, tag="dn", name=f"dn{ti}")
                nc.scalar.activation(dn[:, :, :], pv[HD:HD + 1, :, :],
                                     mybir.ActivationFunctionType.Identity)
                dr0 = drp.tile([1, 4, 128], f32, tag="dr0", name=f"dr0_{ti}")
                nc.vector.reciprocal_approx_fast(dr0[:, :, :], dn[:, :, :])
                dr = drp.tile([1, 4, 128], f32r, tag="dr", name=f"dr{ti}")
                nc.scalar.activation(dr[:, :, :], dr0[:, :, :],
                                     mybir.ActivationFunctionType.Identity)
                if dbg and (hh, g, qb) == (0, 0, 0):
                    nc.sync.dma_start(out=dbg_t["dbg_den"][:, :, :], in_=dr[:, :, :])
                pending = (g, hh, qb, dr)
            emit_norm(pending)

            if dbg:
                nc.sync.dma_start(out=dbg_t["dbg_qT"][:, :, :], in_=qT_sb[:, :, :])
                nc.sync.dma_start(out=dbg_t["dbg_attnT"][:, :, :], in_=attnT[:, :, :])

            # ---- output projection ----
            for do in range(KC):
                ps = psA.tile([128, 512], f32, tag="ps")
                out = ps[:, :]
                for p in range(KC):
                    nc.tensor.matmul(
                        out, wo_sb[:, do, p, :], attnT[:, p, :],
                        start=(p == 0), stop=(p == KC - 1),
                    )
                yt = yst.tile([128, SQ], f32, tag="yt")
                nc.scalar.activation(yt, out, mybir.ActivationFunctionType.Identity,
                                     bias=bo_sb[:, do:do + 1])
                eng = nc.sync if do % 2 == 0 else nc.scalar
                eng.dma_start(out=yT[do * 128:(do + 1) * 128, :], in_=yt[:, :])

    nc.finalize()
    return nc


def get_program(dbg=False):
    key = ("nc", dbg)
    if key not in _CACHE:
        _CACHE[key] = _build_program(dbg)
    return _CACHE[key]


def make_in_maps(x, Wq, bq, Wk, bk, Wv, bv, Wo, bo):
    """Host-side sharding: per-core input dicts."""
    import ml_dtypes

    bf16 = ml_dtypes.bfloat16
    x = np.ascontiguousarray(np.asarray(x, np.float32))
    wkb = np.concatenate([np.asarray(Wk, np.float32), np.asarray(bk, np.float32)[None]], 0)
    wvb = np.concatenate([np.asarray(Wv, np.float32), np.asarray(bv, np.float32)[None]], 0)
    # head permutation: device column-block p holds [head p | head p+8]
    perm = np.empty(DIM, np.int64)
    for p in range(8):
        perm[128 * p:128 * p + 64] = np.arange(64 * p, 64 * p + 64)
        perm[128 * p + 64:128 * p + 128] = np.arange(64 * (p + 8), 64 * (p + 8) + 64)
    wqp = np.asarray(Wq, np.float32)[:, perm]
    wop = np.asarray(Wo, np.float32)[perm, :]
    # [dd, part, kc, m] blocks so each dd's weights are one contiguous DMA
    wq_blk = np.ascontiguousarray(
        wqp.reshape(KC, 128, KC, 128).transpose(2, 1, 0, 3).astype(bf16))
    wo_blk = np.ascontiguousarray(
        wop.reshape(KC, 128, KC, 128).transpose(2, 1, 0, 3).astype(bf16))
    common = {
        "wq": wq_blk,
        "wk": np.ascontiguousarray(wkb.astype(bf16)),
        "wv": np.ascontiguousarray(wvb.astype(bf16)),
        "wo": wo_blk,
        "bq": np.ascontiguousarray(np.asarray(bq, np.float32)[perm].reshape(DIM, 1)),
        "bo": np.ascontiguousarray(np.asarray(bo, np.float32).reshape(DIM, 1)),
        "ident": np.eye(128, dtype=np.float32),
        "ones2": np.ones((128, G), bf16),
        "ones64": np.ones((1, 64), np.float32),
    }
    in_maps = []
    for c in range(NCORES):
        b, t = divmod(c, NCORES // BATCH)
        s0 = SQ * t
        xa = np.zeros((SK, DIM + 1), np.float32)
        lo, hi = max(0, s0 - HALF), min(SEQ, s0 + SQ + HALF)
        xa[lo - (s0 - HALF):hi - (s0 - HALF), :DIM] = x[b, lo:hi]
        xa[lo - (s0 - HALF):hi - (s0 - HALF), DIM] = 1.0
        in_maps.append({"xaT": np.ascontiguousarray(xa.T.astype(bf16)), **common})
    return in_maps


def assemble_output(results):
    y = np.empty((BATCH, SEQ, DIM), np.float32)
    for c in range(NCORES):
        b, t = divmod(c, NCORES // BATCH)
        y[b, SQ * t:SQ * (t + 1), :] = results[c]["yT"].T
    return y


def kernel(**inputs):
    from concourse.bass_utils import run_bass_kernel_spmd

    nc = get_program()
    in_maps = make_in_maps(**inputs)
    last_err = None
    for _ in range(3):  # retry: transient NRT device wedges recover on rerun
        try:
            res = run_bass_kernel_spmd(nc, in_maps, list(range(NCORES)))
            return assemble_output(res.results)
        except Exception as e:  # noqa: BLE001
            last_err = e
    raise last_err


# revision 29
# speedup vs baseline: 1.3272x; 1.1537x over previous
"""GQA sliding-window attention (training path, no causal mask, no 1/sqrt(d)
scaling) on 8 Trainium2 NeuronCores.

Reference semantics (see original nn.Module):
  q = x@Wq+bq [b,s,16,64]; k,v = x@Wk+bk / x@Wv+bv [b,s,2,64]
  k,v zero-padded by 128 on both sides of s; query i attends padded
  positions [i, i+256) (i.e. global [i-128, i+128)); padded positions
  contribute score 0 (exp->1) and value 0. out = attn @ Wo + bo.

Sharding: batch x sequence. 8 shards = 2 batches x 4 chunks of 512 query
rows. Each core receives x^T for its 512 rows plus a 128-row halo on each
side (zero rows outside [0, 2048)), with an appended 0/1 validity row so
that K/V bias is only added at in-range positions. Host gathers per-core
outputs; no collectives.

Per-core dataflow (bf16 matmul inputs, fp32 PSUM accumulation):
  xT -> kT/vT projections (bias via augmented weight row), V transposed
  back to natural layout with a ones column appended (so PV matmuls also
  produce the softmax denominator). qT computed per 128-dim block with
  ScalarE bias-add; host permutes Wq columns so block p holds head p
  (group 0) in rows 0:64 and head p+8 (group 1) in rows 64:128.
  Attention runs in 4-head-packed tiles: per (group, head-half, qblock),
  scores S^T[w 128, 4 heads, 128 q] are one N=512 matmul per window
  chunk (3 chunks cover each qblock's 384-position window), exp on
  ScalarE, band-mask triangles via 2 GPSIMD affine_selects (multi-dim
  pattern broadcasts over the 4 heads), PV accumulates [65, 4, 128]
  (row 64 = denominator). Denominator reciprocal via DVE
  reciprocal_approx_fast directly from PSUM; normalization multiplies
  attnT by the reciprocal broadcast across partitions (stride-0 AP, or
  a ones-outer-product PSUM matmul fallback). Output projection with
  ScalarE bias-add, streamed to DRAM.
"""

import numpy as np

DIM = 1024
NH = 16  # query heads
G = 2  # kv heads
HD = 64  # head dim
W = 256  # window
HALF = 128
BATCH, SEQ = 2, 2048
NCORES = 8
SQ = 512  # query rows per core
SK = SQ + 2 * HALF  # 768 kv halo rows per core
KC = DIM // 128  # 8 contraction chunks
NJ = SK // 128  # 6 kv chunks



_CACHE = {}


def _build_program(dbg=False):
    import concourse.bass as bass
    import concourse.mybir as mybir
    import concourse.tile as tile
    from concourse import bacc

    f32 = mybir.dt.float32
    f32r = mybir.dt.float32r
    bf16 = mybir.dt.bfloat16

    nc = bacc.Bacc("TRN2", target_bir_lowering=False, debug=False, num_devices=NCORES)
    dbg_t = {}
    if dbg:
        for name, shape, dt in [
            ("dbg_qT", [128, KC, SQ], bf16), ("dbg_kT", [128, SK], bf16),
            ("dbg_vT", [128, SK], f32r),
            ("dbg_pt0", [128, 3, 4, 128], bf16),
            ("dbg_attnT", [128, KC, SQ], bf16),
            ("dbg_pvden", [1, 4, 128], f32),
            ("dbg_recs", [1, 4, 128], f32),
        ]:
            dbg_t[name] = nc.declare_dram_parameter(name, shape, dt, isOutput=True)

    xaT = nc.declare_dram_parameter("xaT", [DIM + 1, SK], bf16, isOutput=False)
    wq = nc.declare_dram_parameter("wq", [KC, 128, KC, 128], bf16, isOutput=False)
    wk = nc.declare_dram_parameter("wk", [DIM + 1, G * HD], bf16, isOutput=False)
    wv = nc.declare_dram_parameter("wv", [DIM + 1, G * HD], bf16, isOutput=False)
    wo = nc.declare_dram_parameter("wo", [KC, 128, KC, 128], bf16, isOutput=False)
    bq = nc.declare_dram_parameter("bq", [DIM, 1], f32, isOutput=False)
    bo = nc.declare_dram_parameter("bo", [DIM, 1], f32, isOutput=False)
    identD = nc.declare_dram_parameter("ident", [128, 128], f32r, isOutput=False)
    ones2 = nc.declare_dram_parameter("ones2", [128, G], bf16, isOutput=False)
    triD = nc.declare_dram_parameter("tri", [2, 128, 128], bf16, isOutput=False)
    yT = nc.declare_dram_parameter("yT", [DIM, SQ], f32, isOutput=True)

    with tile.TileContext(nc) as tc:
        with (
            nc.allow_low_precision("bf16 matmul inputs; accumulation stays fp32"),
            tc.tile_pool(name="wts", bufs=1) as wts,
            tc.tile_pool(name="sb", bufs=1) as sb,
            tc.tile_pool(name="pt", bufs=3) as ptp,
            tc.tile_pool(name="dr", bufs=3) as drp,
            tc.tile_pool(name="rbp", bufs=2) as rbp,
            tc.tile_pool(name="yst", bufs=2) as yst,
            tc.tile_pool(name="psS", bufs=2, space="PSUM") as psS,
            tc.tile_pool(name="pvP", bufs=2, space="PSUM") as pvP,
        ):
            # ---- constant loads (critical-path order: wk/wv, xT, wq, wo) ----
            wk_sb = wts.tile([128, KC, G * HD], bf16, tag="wk")
            wv_sb = wts.tile([128, KC, G * HD], bf16, tag="wv")
            nc.sync.dma_start(
                out=wk_sb[:, :, :],
                in_=wk[0:DIM, :].rearrange("(kc p) m -> p kc m", p=128))
            nc.scalar.dma_start(
                out=wv_sb[:, :, :],
                in_=wv[0:DIM, :].rearrange("(kc p) m -> p kc m", p=128))
            xT_sb = wts.tile([128, KC, SK], bf16, tag="xT")
            for kc in range(KC):
                eng = nc.sync if kc % 2 == 0 else nc.scalar
                eng.dma_start(out=xT_sb[:, kc, :], in_=xaT[kc * 128:(kc + 1) * 128, :])
            wq_sb = wts.tile([128, KC, KC, 128], bf16, tag="wq")
            for dd in range(KC):
                eng = nc.sync if dd % 2 == 0 else nc.scalar
                eng.dma_start(out=wq_sb[:, dd, :, :], in_=wq[dd, :, :, :])
            wo_sb = wts.tile([128, KC, KC, 128], bf16, tag="wo")
            for do in range(KC):
                eng = nc.sync if do % 2 == 0 else nc.scalar
                eng.dma_start(out=wo_sb[:, do, :, :], in_=wo[do, :, :, :])

            xaug = wts.tile([1, SK], bf16, tag="xaug")
            nc.gpsimd.dma_start(out=xaug[:, :], in_=xaT[DIM:DIM + 1, :])
            wk_aug = wts.tile([1, G * HD], bf16, tag="wkaug")
            wv_aug = wts.tile([1, G * HD], bf16, tag="wvaug")
            nc.gpsimd.dma_start(out=wk_aug[:, :], in_=wk[DIM:DIM + 1, :])
            nc.gpsimd.dma_start(out=wv_aug[:, :], in_=wv[DIM:DIM + 1, :])
            bq_sb = wts.tile([128, KC], f32, tag="bq")
            bo_sb = wts.tile([128, KC], f32, tag="bo")
            nc.gpsimd.dma_start(
                out=bq_sb[:, :], in_=bq.rearrange("(a p) c -> p (a c)", p=128))
            nc.gpsimd.dma_start(
                out=bo_sb[:, :], in_=bo.rearrange("(a p) c -> p (a c)", p=128))
            ident = wts.tile([128, 128], f32r, tag="ident")
            nc.gpsimd.dma_start(out=ident[:, :], in_=identD[:, :])
            ones_sb = wts.tile([128, G], bf16, tag="ones")
            nc.gpsimd.dma_start(out=ones_sb[:, :], in_=ones2[:, :])
            tri_sb = wts.tile([128, 2, 128], bf16, tag="tri")
            nc.gpsimd.dma_start(
                out=tri_sb[:, :, :], in_=triD.rearrange("t p m -> p t m"))

            # ---- persistent intermediates ----
            qT_sb = sb.tile([128, KC, SQ], bf16, tag="qT")  # [dk(2 heads), dd, q]
            kT_sb = sb.tile([128, SK], bf16, tag="kT")      # [dk(2 groups), w]
            vT_sb = sb.tile([128, SK], f32r, tag="vT")
            vt_t = [
                sb.tile([128, G, HD + 1], bf16, tag=f"vt{j}", name=f"vt{j}")
                for j in range(NJ)
            ]
            attnT = sb.tile([128, KC, SQ], bf16, tag="attnT")  # [dk(2 heads), pair, q]

            # ---- K/V projections over the full 768 halo ----
            # The tiny aug bias-row matmul goes FIRST (its operands arrive on
            # the fast gpsimd queue) so the PE starts before wk/xT land.
            for (wmat, waug, dst) in ((wk_sb, wk_aug, kT_sb), (wv_sb, wv_aug, vT_sb)):
                for h2 in range(2):
                    ps = psS.tile([128, KC, 128], f32, tag="sc")
                    out = ps[:, 0:3, :].rearrange("p a b -> p (a b)")
                    sl = slice(h2 * 384, (h2 + 1) * 384)
                    nc.tensor.matmul(out, waug[:, :], xaug[:, sl],
                                     start=True, stop=False)
                    for kc in range(KC):
                        nc.tensor.matmul(
                            out, wmat[:, kc, :], xT_sb[:, kc, sl],
                            start=False, stop=(kc == KC - 1),
                        )
                    nc.vector.tensor_copy(dst[:, sl], out)

            # ---- V back to natural layout [w, dk], ones column appended ----
            for j in range(NJ):
                ps = psS.tile([128, KC, 128], f32r, tag="sc", name=f"pstr{j}")
                out = ps[:, 0, :]
                nc.tensor.transpose(out, vT_sb[:, j * 128:(j + 1) * 128], ident)
                nc.vector.tensor_copy(
                    vt_t[j][:, :, 0:HD],
                    out.rearrange("p (g d) -> p g d", g=G),
                )
                nc.vector.tensor_copy(vt_t[j][:, :, HD:HD + 1], ones_sb[:, :])

            def q_proj(dd):
                ps = psS.tile([128, KC, 128], f32, tag="sc", name=f"psq{dd}")
                out = ps[:, 0:4, :].rearrange("p a b -> p (a b)")
                for kc in range(KC):
                    nc.tensor.matmul(
                        out, wq_sb[:, dd, kc, :], xT_sb[:, kc, HALF:HALF + SQ],
                        start=(kc == 0), stop=(kc == KC - 1),
                    )
                nc.scalar.activation(
                    qT_sb[:, dd, :], out, mybir.ActivationFunctionType.Identity,
                    bias=bq_sb[:, dd:dd + 1],
                )

            if dbg:
                nc.sync.dma_start(out=dbg_t["dbg_kT"][:, :], in_=kT_sb[:, :])
                nc.sync.dma_start(out=dbg_t["dbg_vT"][:, :], in_=vT_sb[:, :])

            # ---- attention in 8-head pair-tiles ----
            # Tile (g, qb): all 8 heads of group g, local q block
            # [128qb, 128qb+128). Window chunks qb+c (c=0..2); band masks:
            # c=0 keeps ww-qq>=0, c=2 keeps qq-ww-1>=0, c=1 is full. Scores
            # and PV run as hh-half matmuls (moving free dim caps at 512)
            # into shared [128, 8, 128] PSUM tiles.
            for dd in range(KC):
                q_proj(dd)

            tiles = [(g, qb) for g in range(G) for qb in range(4)]
            pending = None

            def emit_norm(p):
                # deferred one tile so the GpSimd broadcast + DVE multiply
                # never stall the PE stream
                g, qb, dr, ti = p
                region = attnT[64 * g:64 * g + 64, :,
                               qb * 128:(qb + 1) * 128]
                # both SBUF operands of tensor_tensor must share their base
                # partition: place the broadcast at the same 64g offset
                rb = rbp.tile([128, KC, 128], bf16, tag="rb", name=f"rb{ti}")
                nc.gpsimd.partition_broadcast(rb[:, :, :], dr[:, :, :])
                nc.vector.tensor_mul(region, region,
                                     rb[64 * g:64 * g + 64, :, :])

            for ti, (g, qb) in enumerate(tiles):
                pt = ptp.tile([128, 3, KC, 128], bf16, tag="pt")
                for c in range(3):
                    ps = psS.tile([128, KC, 128], f32, tag="sc")
                    for hh in range(2):
                        nc.tensor.matmul(
                            ps[:, 4 * hh:4 * hh + 4, :],
                            kT_sb[64 * g:64 * g + 64,
                                  (qb + c) * 128:(qb + c + 1) * 128],
                            qT_sb[64 * g:64 * g + 64, 4 * hh:4 * hh + 4,
                                  qb * 128:(qb + 1) * 128],
                            start=True, stop=True,
                        )
                    nc.scalar.activation(pt[:, c, :, :], ps[:, :, :],
                                         mybir.ActivationFunctionType.Exp)
                    if c == 0:
                        nc.gpsimd.affine_select(
                            out=pt[:, 0, :, :], in_=pt[:, 0, :, :],
                            compare_op=mybir.AluOpType.is_ge, fill=0.0,
                            base=0, channel_multiplier=1,
                            pattern=[[0, KC], [-1, 128]],
                        )
                    elif c == 2:
                        nc.gpsimd.affine_select(
                            out=pt[:, 2, :, :], in_=pt[:, 2, :, :],
                            compare_op=mybir.AluOpType.is_ge, fill=0.0,
                            base=-1, channel_multiplier=-1,
                            pattern=[[0, KC], [1, 128]],
                        )
                if dbg and (g, qb) == (0, 0):
                    nc.sync.dma_start(out=dbg_t["dbg_pt0"][:, :, :, :],
                                      in_=pt[:, :, 0:4, :])
                pv = pvP.tile([128, KC, 128], f32, tag="pv")
                for c in range(3):
                    for hh in range(2):
                        nc.tensor.matmul(
                            pv[0:HD + 1, 4 * hh:4 * hh + 4, :],
                            vt_t[qb + c][:, g, :],
                            pt[:, c, 4 * hh:4 * hh + 4, :],
                            start=(c == 0), stop=(c == 2),
                        )
                if pending is not None:
                    emit_norm(pending)
                nc.vector.tensor_copy(
                    attnT[64 * g:64 * g + 64, :, qb * 128:(qb + 1) * 128],
                    pv[0:HD, :, :])
                # reciprocal_approx_fast cannot read PSUM: ScalarE moves the
                # denominator row to SBUF, DVE approximates 1/x and casts to
                # bf16 for the broadcast.
                dn = drp.tile([1, KC, 128], f32, tag="dn", name=f"dn{ti}")
                nc.scalar.activation(dn[:, :, :], pv[HD:HD + 1, :, :],
                                     mybir.ActivationFunctionType.Identity)
                if dbg and (g, qb) == (0, 0):
                    nc.sync.dma_start(out=dbg_t["dbg_pvden"][:, :, :],
                                      in_=dn[:, 0:4, :])
                dr0 = drp.tile([1, KC, 128], f32, tag="dr0", name=f"dr0_{ti}")
                nc.vector.reciprocal_approx_fast(dr0[:, :, :], dn[:, :, :])
                dr = drp.tile([1, KC, 128], bf16, tag="dr", name=f"dr{ti}")
                nc.vector.tensor_copy(dr[:, :, :], dr0[:, :, :])
                if dbg and (g, qb) == (0, 0):
                    nc.sync.dma_start(out=dbg_t["dbg_recs"][:, :, :],
                                      in_=dr0[:, 0:4, :])
                pending = (g, qb, dr, ti)
            emit_norm(pending)

            if dbg:
                nc.sync.dma_start(out=dbg_t["dbg_qT"][:, :, :], in_=qT_sb[:, :, :])
                nc.sync.dma_start(out=dbg_t["dbg_attnT"][:, :, :], in_=attnT[:, :, :])

            # ---- output projection ----
            for do in range(KC):
                ps = psS.tile([128, KC, 128], f32, tag="sc")
                out = ps[:, 0:4, :].rearrange("p a b -> p (a b)")
                for p in range(KC):
                    nc.tensor.matmul(
                        out, wo_sb[:, do, p, :], attnT[:, p, :],
                        start=(p == 0), stop=(p == KC - 1),
                    )
                yt = yst.tile([128, SQ], f32, tag="yt")
                nc.scalar.activation(yt, out, mybir.ActivationFunctionType.Identity,
                                     bias=bo_sb[:, do:do + 1])
                eng = nc.sync if do % 2 == 0 else nc.scalar
                eng.dma_start(out=yT[do * 128:(do + 1) * 128, :], in_=yt[:, :])

    nc.finalize()
    return nc


def get_program(dbg=False):
    key = ("nc", dbg)
    if key not in _CACHE:
        _CACHE[key] = _build_program(dbg)
    return _CACHE[key]


def make_in_maps(x, Wq, bq, Wk, bk, Wv, bv, Wo, bo):
    """Host-side sharding: per-core input dicts."""
    import ml_dtypes

    bf16 = ml_dtypes.bfloat16
    x = np.ascontiguousarray(np.asarray(x, np.float32))
    wkb = np.concatenate([np.asarray(Wk, np.float32), np.asarray(bk, np.float32)[None]], 0)
    wvb = np.concatenate([np.asarray(Wv, np.float32), np.asarray(bv, np.float32)[None]], 0)
    # head permutation: device column-block p holds [head p | head p+8]
    perm = np.empty(DIM, np.int64)
    for p in range(8):
        perm[128 * p:128 * p + 64] = np.arange(64 * p, 64 * p + 64)
        perm[128 * p + 64:128 * p + 128] = np.arange(64 * (p + 8), 64 * (p + 8) + 64)
    wqp = np.asarray(Wq, np.float32)[:, perm]
    wop = np.asarray(Wo, np.float32)[perm, :]
    # [dd, part, kc, m] blocks so each dd's weights are one contiguous DMA
    wq_blk = np.ascontiguousarray(
        wqp.reshape(KC, 128, KC, 128).transpose(2, 1, 0, 3).astype(bf16))
    wo_blk = np.ascontiguousarray(
        wop.reshape(KC, 128, KC, 128).transpose(2, 1, 0, 3).astype(bf16))
    common = {
        "wq": wq_blk,
        "wk": np.ascontiguousarray(wkb.astype(bf16)),
        "wv": np.ascontiguousarray(wvb.astype(bf16)),
        "wo": wo_blk,
        "bq": np.ascontiguousarray(np.asarray(bq, np.float32)[perm].reshape(DIM, 1)),
        "bo": np.ascontiguousarray(np.asarray(bo, np.float32).reshape(DIM, 1)),
        "ident": np.eye(128, dtype=np.float32),
        "ones2": np.ones((128, G), bf16),
        "tri": np.stack([
            (np.arange(128)[:, None] >= np.arange(128)[None, :]),
            (np.arange(128)[None, :] > np.arange(128)[:, None]),
        ]).astype(bf16),
    }
    in_maps = []
    for c in range(NCORES):
        b, t = divmod(c, NCORES // BATCH)
        s0 = SQ * t
        xa = np.zeros((SK, DIM + 1), np.float32)
        lo, hi = max(0, s0 - HALF), min(SEQ, s0 + SQ + HALF)
        xa[lo - (s0 - HALF):hi - (s0 - HALF), :DIM] = x[b, lo:hi]
        xa[lo - (s0 - HALF):hi - (s0 - HALF), DIM] = 1.0
        in_maps.append({"xaT": np.ascontiguousarray(xa.T.astype(bf16)), **common})
    return in_maps


def assemble_output(results):
    y = np.empty((BATCH, SEQ, DIM), np.float32)
    for c in range(NCORES):
        b, t = divmod(c, NCORES // BATCH)
        y[b, SQ * t:SQ * (t + 1), :] = results[c]["yT"].T
    return y


def kernel(**inputs):
    from concourse.bass_utils import run_bass_kernel_spmd

    nc = get_program()
    in_maps = make_in_maps(**inputs)
    last_err = None
    for _ in range(3):  # retry: transient NRT device wedges recover on rerun
        try:
            res = run_bass_kernel_spmd(nc, in_maps, list(range(NCORES)))
            return assemble_output(res.results)
        except Exception as e:  # noqa: BLE001
            last_err = e
    raise last_err
